# revision 18
# baseline (speedup 1.0000x reference)
"""VQ-codebook model on 8 Trainium2 NeuronCores (Bass/Tile), data-parallel over batch.

Per core (64 batches): embedding gather (indirect DMA) -> cosine-sim vs codebook
(TensorE matmuls, norms folded into operands) -> Conv1d(A->A, k=51) as 51x2
shifted matmul accumulations in PSUM (bf16 weights/acts, fp32 accum) -> ReLU +
channel-max (partition max-tree) -> masked softmax over sequence -> z_s matvecs
-> p_t_ / p_t / r_s small matmuls. No collectives needed (pure data parallel).
"""

import sys
import numpy as np

for _p in ("/opt/trn_rl_repo", "/root/.axon_site/_ro/trn_rl_repo"):
    if _p not in sys.path:
        sys.path.append(_p)

B, S, E, A, V, K = 512, 200, 512, 256, 50000, 51
PAD = 1
NEG_INF = -1e13
N_CORES = 8
B_LOC = B // N_CORES          # 64 batches per core
BG = 4                        # batches per pipelined group
N_GROUPS = B_LOC // BG        # 16
HALF_K = K // 2               # 25
SPAD = S + 2 * HALF_K         # 250 padded sequence length in g buffer

_CACHE = {}


def _build(n_groups=N_GROUPS):
    import concourse.bass as bass
    import concourse.bacc as bacc
    import concourse.mybir as mybir
    import concourse.tile as tile
    from concourse.masks import make_identity
    from concourse import bass_isa

    dt = mybir.dt
    f32 = dt.float32
    f32r = dt.float32r
    bf16 = dt.bfloat16
    i32 = dt.int32
    AX = mybir.AxisListType
    AF = mybir.ActivationFunctionType
    OP = mybir.AluOpType

    nc = bacc.Bacc(None, target_bir_lowering=False)

    emb_d = nc.dram_tensor("emb", [V, E], f32, kind="ExternalInput")
    t_d = nc.dram_tensor("t_mat", [A, E], f32, kind="ExternalInput")
    ww_d = nc.dram_tensor("w_w", [A, E], f32, kind="ExternalInput")
    wb_d = nc.dram_tensor("w_b", [1, A], f32, kind="ExternalInput")
    cw_d = nc.dram_tensor("conv_w", [A, A * K], f32, kind="ExternalInput")
    cb_d = nc.dram_tensor("conv_b", [1, A], f32, kind="ExternalInput")
    # x in two layouts, prepared on host (layout prep only):
    #   xidx[p, h, b] = x[b, h*128 + p]   (gather indices, token-on-partition)
    #   xrow[b, s]    = x[b, s]           (mask rows, batch-on-partition)
    xidx_d = nc.dram_tensor("x_idx", [128, 2, B_LOC], i32, kind="ExternalInput")
    xrow_d = nc.dram_tensor("x_row", [B_LOC, S], i32, kind="ExternalInput")

    pt_d = nc.dram_tensor("p_t", [B_LOC, A], f32, kind="ExternalOutput")
    zs_d = nc.dram_tensor("z_s", [B_LOC, E], f32, kind="ExternalOutput")
    rs_d = nc.dram_tensor("r_s", [B_LOC, E], f32, kind="ExternalOutput")
    ai_d = nc.dram_tensor("a_i", [B_LOC, S], f32, kind="ExternalOutput")
    ptl_d = nc.dram_tensor("p_t_", [B_LOC, A], f32, kind="ExternalOutput")

    with tile.TileContext(nc) as tc:
        with (
            tc.tile_pool(name="pers", bufs=1) as pers,
            tc.tile_pool(name="stage", bufs=2) as stage,
            tc.tile_pool(name="xx", bufs=5) as xxp,
            tc.tile_pool(name="scr", bufs=2) as scr,
            tc.tile_pool(name="xxt", bufs=2) as xxtp,
            tc.tile_pool(name="gbuf", bufs=2) as gbp,
            tc.tile_pool(name="small", bufs=2) as smp,
            tc.tile_pool(name="ppsum", bufs=2, space="PSUM") as ppsum,
            tc.tile_pool(name="mpsum", bufs=2, space="PSUM") as mpsum,
            tc.tile_pool(name="tpsum", bufs=2, space="PSUM") as tpsum,
        ):
            # ---------------- init: constants, weights, transposes -------------
            ident = pers.tile([128, 128], f32)
            make_identity(nc, ident[:])
            identb = pers.tile([128, 128], bf16)
            nc.vector.tensor_copy(identb[:], ident[:])

            ones_row = pers.tile([1, 128], f32)
            nc.vector.memset(ones_row[:], 1.0)
            neg_row = pers.tile([2, S], f32)
            nc.vector.memset(neg_row[:], NEG_INF)
            eps_col = pers.tile([128, 1], f32)
            nc.vector.memset(eps_col[:], 1e-26)

            # x index / row loads
            xidx = pers.tile([128, 2, B_LOC], i32)
            nc.sync.dma_start(out=xidx[:], in_=xidx_d[:])


            # conv bias as per-partition scalars for the two o-chunks
            cb_sb = pers.tile([128, 2], f32)
            nc.sync.dma_start(
                out=cb_sb[:], in_=cb_d[:].rearrange("o (h p) -> (o p) h", h=2, p=128)
            )

            # W_b broadcast to all partitions via rank-1 matmul
            wb_row = pers.tile([1, A], f32)
            nc.sync.dma_start(out=wb_row[:], in_=wb_d[:])
            wb_ps = tpsum.tile([128, A], f32, tag="tp")
            nc.tensor.matmul(wb_ps[:], ones_row[:], wb_row[:], start=True, stop=True)
            wb_bc = pers.tile([128, A], f32)
            nc.vector.tensor_copy(wb_bc[:], wb_ps[:])

            # T rows (unscaled, rhs of r_s) + normalized-transposed T for cosine
            t_rows = pers.tile([128, 2, E], f32)
            nc.sync.dma_start(
                out=t_rows[:], in_=t_d[:].rearrange("(c p) e -> p c e", c=2, p=128)
            )
            t_t = pers.tile([128, 4, A], bf16)  # [e-part, e-chunk, a] scaled by 1/|T_a|
            sq_scr = stage.tile([128, E], f32, tag="sqscr")
            for c in range(2):
                n2 = smp.tile([128, 1], f32, tag="n2")
                nc.scalar.activation(sq_scr[:], t_rows[:, c, :], AF.Square, accum_out=n2[:])
                nrm = smp.tile([128, 1], f32, tag="nrm")
                nc.scalar.activation(nrm[:], n2[:], AF.Sqrt, bias=eps_col[:])
                inv = smp.tile([128, 1], f32, tag="inv")
                nc.vector.reciprocal(inv[:], nrm[:])
                t_nrm = stage.tile([128, E], bf16, tag="tnrm")
                nc.vector.tensor_scalar_mul(t_nrm[:], t_rows[:, c, :], inv[:])
                for ec in range(4):
                    psb = tpsum.tile([128, 128], bf16, tag="tp")
                    nc.tensor.transpose(psb[:], t_nrm[:, ec * 128:(ec + 1) * 128], identb[:])
                    nc.vector.tensor_copy(t_t[:, ec, c * 128:(c + 1) * 128], psb[:])

            # W_w transposed [e, a]
            ww_t = pers.tile([128, 4, A], f32)
            for c in range(2):
                ww_rows = stage.tile([128, E], f32, tag="wwrow")
                nc.sync.dma_start(out=ww_rows[:], in_=ww_d[c * 128:(c + 1) * 128, :])
                for ec in range(4):
                    ps = tpsum.tile([128, 128], f32, tag="tp")
                    nc.tensor.transpose(ps[:], ww_rows[:, ec * 128:(ec + 1) * 128], ident[:])
                    nc.vector.tensor_copy(ww_t[:, ec, c * 128:(c + 1) * 128], ps[:])

            # conv weights: [o, i, k] rows -> cw_t[ic][i_local, k, o] in bf16
            cw_t = [pers.tile([128, K, A], bf16, name=f"cw_t{i}", tag=f"cw_t{i}") for i in range(2)]
            for oc in range(2):
                for iq in range(4):
                    ic, qh = iq // 2, iq % 2
                    cw_rows = stage.tile([128, 64 * K], f32, tag="cwrow")
                    nc.sync.dma_start(
                        out=cw_rows[:],
                        in_=cw_d[oc * 128:(oc + 1) * 128, iq * 64 * K:(iq + 1) * 64 * K],
                    )
                    cw_v = cw_rows[:].rearrange("p (il k) -> p il k", il=64, k=K)
                    for k in range(K):
                        ps = tpsum.tile([128, 128], f32, tag="tp")
                        nc.tensor.transpose(
                            ps[0:64, :], cw_v[:, :, k], ident[:]
                        )
                        nc.vector.tensor_copy(
                            cw_t[ic][qh * 64:(qh + 1) * 64, k, oc * 128:(oc + 1) * 128],
                            ps[0:64, :],
                        )

            # persistent accumulator for z (needed on-chip by the tail)
            z_all = pers.tile([B_LOC, E], f32)
            nc.vector.memset(z_all[:], 0.0)

            # ---------------- main loop over groups of BG batches ---------------
            for gi in range(n_groups):
                # group g buffer (conv input), bf16, zero margins
                g_g = [gbp.tile([128, BG, SPAD], bf16, name=f"g_{ic}", tag=f"g{ic}") for ic in range(2)]
                for ic in range(2):
                    nc.vector.memset(g_g[ic][:, :, 0:HALF_K], 0.0)
                    nc.vector.memset(g_g[ic][:, :, S + HALF_K:SPAD], 0.0)

                xxt = xxtp.tile([128, 4, BG * S], bf16, tag="xxt")
                xx_b = []
                for bi in range(BG):
                    b = gi * BG + bi
                    # gather embedding rows: tokens [b*200, b*200+200)
                    xx = xxp.tile([128, 2 * E], f32, tag="xx")
                    nc.gpsimd.indirect_dma_start(
                        out=xx[:, 0:E], out_offset=None, in_=emb_d[:],
                        in_offset=bass.IndirectOffsetOnAxis(ap=xidx[:, 0, b:b + 1], axis=0),
                    )
                    nc.gpsimd.indirect_dma_start(
                        out=xx[0:72, E:2 * E], out_offset=None, in_=emb_d[:],
                        in_offset=bass.IndirectOffsetOnAxis(ap=xidx[0:72, 1, b:b + 1], axis=0),
                    )
                    xxb = xxp.tile([128, 2 * E], bf16, tag="xxb")
                    xx_b.append(xxb)
                    nc.vector.tensor_copy(xxb[:, 0:E], xx[:, 0:E])
                    nc.vector.tensor_copy(xxb[0:72, E:2 * E], xx[0:72, E:2 * E])
                    # token norms -> 1/|xx_s| ; scale rows, then transpose per e-chunk
                    for h, np_ in ((0, 128), (1, 72)):
                        sq = scr.tile([128, E], f32, tag="sq")
                        n2 = smp.tile([128, 1], f32, tag="n2")
                        nc.vector.tensor_tensor(
                            sq[0:np_, :], xx[0:np_, h * E:(h + 1) * E],
                            xx[0:np_, h * E:(h + 1) * E], op=OP.mult,
                        )
                        nc.vector.tensor_reduce(
                            n2[0:np_, :], sq[0:np_, :], axis=AX.X, op=OP.add,
                        )
                        nrm = smp.tile([128, 1], f32, tag="nrm")
                        nc.scalar.activation(nrm[0:np_, :], n2[0:np_, :], AF.Sqrt, bias=eps_col[0:np_, :])
                        inv = smp.tile([128, 1], f32, tag="inv")
                        nc.vector.reciprocal(inv[0:np_, :], nrm[0:np_, :])
                        xs = scr.tile([128, E], bf16, tag="xs")
                        nc.vector.tensor_scalar_mul(
                            xs[0:np_, :], xx[0:np_, h * E:(h + 1) * E], inv[0:np_, :]
                        )
                        for ec in range(4):
                            psb = tpsum.tile([128, 128], bf16, tag="tp")
                            nc.tensor.transpose(
                                psb[0:128, 0:np_], xs[0:np_, ec * 128:(ec + 1) * 128],
                                identb[0:np_, 0:np_],
                            )
                            nc.vector.tensor_copy(
                                xxt[:, ec, bi * S + h * 128: bi * S + h * 128 + np_],
                                psb[:, 0:np_],
                            )

                # cosine similarity -> normalized g, cast to bf16 into g buffer
                for pair in range(2):
                    for ac in range(2):
                        ps = mpsum.tile([128, 2, S], f32, tag="mm")
                        for ec in range(4):
                            nc.tensor.matmul(
                                ps[:],
                                t_t[:, ec, ac * 128:(ac + 1) * 128],
                                xxt[:, ec, pair * 2 * S:(pair + 1) * 2 * S].rearrange(
                                    "p (b s) -> p b s", b=2, s=S
                                ),
                                start=(ec == 0), stop=(ec == 3),
                            )
                        nc.vector.tensor_copy(
                            g_g[ac][:, pair * 2:(pair + 1) * 2, HALF_K:HALF_K + S], ps[:]
                        )

                # conv + post-processing, pair-sequential so psum slots
                # double-buffer across pairs/groups (keeps PE stream dense)
                for pair in range(2):
                    cpsp = [ppsum.tile([128, 2, S], f32, name=f"cps{oc}", tag=f"cv{oc}")
                            for oc in range(2)]
                    for oc in range(2):
                        for k in range(K):
                            for ic in range(2):
                                nc.tensor.matmul(
                                    cpsp[oc][:],
                                    cw_t[ic][:, k, oc * 128:(oc + 1) * 128],
                                    g_g[ic][:, pair * 2:(pair + 1) * 2, k:k + S],
                                    start=(k == 0 and ic == 0),
                                    stop=(k == K - 1 and ic == 1),
                                )
                    u0 = smp.tile([128, 2, S], f32, tag="u0")
                    u1 = smp.tile([128, 2, S], f32, tag="u1")
                    nc.scalar.activation(u0[:], cpsp[0][:], AF.Relu, bias=cb_sb[:, 0:1])
                    nc.scalar.activation(u1[:], cpsp[1][:], AF.Relu, bias=cb_sb[:, 1:2])
                    # channel max: combine the two o-chunks elementwise (same
                    # base partition), then transpose to [s, o] and reduce free
                    nc.vector.tensor_max(u0[:], u0[:], u1[:])
                    ut = smp.tile([128, 2, 2, 128], f32, tag="ut")
                    for j in range(2):
                        for h, sb in ((0, 128), (1, 72)):
                            ps = tpsum.tile([128, 128], f32, tag="tp")
                            nc.tensor.transpose(
                                ps[0:sb, 0:128], u0[:, j, h * 128:h * 128 + sb],
                                ident[:],
                            )
                            nc.vector.tensor_copy(ut[0:sb, h, j, :], ps[0:sb, :])
                    mcols = smp.tile([128, 2, 2], f32, tag="mcols")
                    for j in range(2):
                        for h, sb in ((0, 128), (1, 72)):
                            nc.vector.tensor_reduce(
                                mcols[0:sb, h, j:j + 1], ut[0:sb, h, j, :],
                                axis=AX.X, op=OP.max,
                            )
                    mg = smp.tile([2, S], f32, tag="mg")
                    psm0 = tpsum.tile([128, 128], f32, tag="tp")
                    nc.tensor.transpose(psm0[0:2, 0:128], mcols[:, 0, :], ident[:])
                    nc.vector.tensor_copy(mg[:, 0:128], psm0[0:2, 0:128])
                    psm1 = tpsum.tile([128, 128], f32, tag="tp")
                    nc.tensor.transpose(psm1[0:2, 0:72], mcols[0:72, 1, :], ident[0:72, 0:72])
                    nc.vector.tensor_copy(mg[:, 128:S], psm1[0:2, 0:72])

                    # masked softmax over s for the two batches (rows 0-1)
                    b0 = gi * BG + pair * 2
                    xfi = smp.tile([2, S], i32, tag="xfi")
                    nc.sync.dma_start(out=xfi[:], in_=xrow_d[b0:b0 + 2, :])
                    msk = smp.tile([2, S], dt.uint8, tag="msk")
                    nc.vector.tensor_scalar(
                        msk[:], xfi[:], PAD, None, op0=OP.is_equal
                    )
                    nc.vector.copy_predicated(mg[:], msk[:], neg_row[0:2, :])
                    rmx = smp.tile([2, 1], f32, tag="rmx")
                    nc.vector.tensor_reduce(
                        rmx[:], mg[:], axis=AX.X, op=OP.max, negate=True
                    )
                    em = smp.tile([2, S], f32, tag="em")
                    nc.scalar.activation(em[:], mg[:], AF.Exp, bias=rmx[:])
                    rsm = smp.tile([2, 1], f32, tag="rsm")
                    nc.vector.tensor_reduce(rsm[:], em[:], axis=AX.X, op=OP.add)
                    rin = smp.tile([2, 1], f32, tag="rin")
                    nc.vector.reciprocal(rin[:], rsm[:])
                    ag = smp.tile([2, S], f32, tag="ag")
                    nc.vector.tensor_scalar_mul(ag[:], em[:], rin[:])
                    nc.sync.dma_start(out=ai_d[b0:b0 + 2, :], in_=ag[:])

                    # a_i columns for z_s matvecs
                    at = smp.tile([128, 2, 2], bf16, tag="at")
                    psa0 = tpsum.tile([128, 128], f32, tag="tp")
                    nc.tensor.transpose(psa0[0:128, 0:2], ag[:, 0:128], ident[0:2, 0:2])
                    nc.vector.tensor_copy(at[:, 0, :], psa0[:, 0:2])
                    psa1 = tpsum.tile([128, 128], f32, tag="tp")
                    nc.tensor.transpose(psa1[0:72, 0:2], ag[:, 128:S], ident[0:2, 0:2])
                    nc.vector.tensor_copy(at[0:72, 1, :], psa1[0:72, 0:2])

                    for j in range(2):
                        bi = pair * 2 + j
                        b = gi * BG + bi
                        zp = mpsum.tile([1, E], f32, tag="mm")
                        nc.tensor.matmul(
                            zp[:], at[:, 0, j:j + 1], xx_b[bi][:, 0:E],
                            start=True, stop=False,
                        )
                        nc.tensor.matmul(
                            zp[:], at[0:72, 1, j:j + 1], xx_b[bi][0:72, E:2 * E],
                            start=False, stop=True,
                        )
                        zrow = smp.tile([1, E], f32, tag="zrow")
                        nc.vector.tensor_copy(zrow[:], zp[:])
                        nc.sync.dma_start(out=z_all[b:b + 1, :], in_=zrow[:])

            # ---------------- tail: p_t_, p_t, r_s -----------------------------
            z_t = pers.tile([128, 4, B_LOC], f32)
            for ec in range(4):
                ps = tpsum.tile([128, 128], f32, tag="tp")
                nc.tensor.transpose(
                    ps[0:128, 0:B_LOC], z_all[:, ec * 128:(ec + 1) * 128],
                    ident[0:B_LOC, 0:B_LOC],
                )
                nc.vector.tensor_copy(z_t[:, ec, :], ps[:, 0:B_LOC])
            pp = mpsum.tile([B_LOC, A], f32, tag="mm")
            for ec in range(4):
                nc.tensor.matmul(
                    pp[:], z_t[:, ec, :], ww_t[:, ec, :],
                    start=(ec == 0), stop=(ec == 3),
                )
            ptl = pers.tile([B_LOC, A], f32)
            nc.vector.tensor_add(ptl[:], pp[:], wb_bc[0:B_LOC, :])
            rmx = pers.tile([B_LOC, 1], f32)
            nc.vector.tensor_reduce(rmx[:], ptl[:], axis=AX.X, op=OP.max, negate=True)
            em = pers.tile([B_LOC, A], f32)
            nc.scalar.activation(em[:], ptl[:], AF.Exp, bias=rmx[:])
            rsm = pers.tile([B_LOC, 1], f32)
            nc.vector.tensor_reduce(rsm[:], em[:], axis=AX.X, op=OP.add)
            rin = pers.tile([B_LOC, 1], f32)
            nc.vector.reciprocal(rin[:], rsm[:])
            ptp = pers.tile([B_LOC, A], f32)
            nc.vector.tensor_scalar_mul(ptp[:], em[:], rin[:])

            pt_t = pers.tile([128, 2, B_LOC], f32)
            for ac in range(2):
                ps = tpsum.tile([128, 128], f32, tag="tp")
                nc.tensor.transpose(
                    ps[0:128, 0:B_LOC], ptp[:, ac * 128:(ac + 1) * 128],
                    ident[0:B_LOC, 0:B_LOC],
                )
                nc.vector.tensor_copy(pt_t[:, ac, :], ps[:, 0:B_LOC])
            rp = mpsum.tile([B_LOC, E], f32, tag="mm")
            for ac in range(2):
                nc.tensor.matmul(
                    rp[:], pt_t[:, ac, :], t_rows[:, ac, :],
                    start=(ac == 0), stop=(ac == 1),
                )
            rsb = pers.tile([B_LOC, E], f32)
            nc.vector.tensor_copy(rsb[:], rp[:])

            nc.sync.dma_start(out=pt_d[:], in_=ptp[:])
            nc.sync.dma_start(out=zs_d[:], in_=z_all[:])
            nc.sync.dma_start(out=rs_d[:], in_=rsb[:])
            nc.sync.dma_start(out=ptl_d[:], in_=ptl[:])

    nc.compile()
    return nc


def _get_nc():
    if "nc" not in _CACHE:
        _CACHE["nc"] = _build()
    return _CACHE["nc"]


def _prep_in_map(x, emb_weight, T, W_w, W_b, conv_w, conv_b, core):
    xs = np.asarray(x[core * B_LOC:(core + 1) * B_LOC], dtype=np.int64)
    xi = np.zeros((128, 2, B_LOC), dtype=np.int32)
    xi[:, 0, :] = xs[:, 0:128].T.astype(np.int32)
    xi[0:72, 1, :] = xs[:, 128:200].T.astype(np.int32)
    return {
        "emb": np.ascontiguousarray(emb_weight, dtype=np.float32),
        "t_mat": np.ascontiguousarray(T, dtype=np.float32),
        "w_w": np.ascontiguousarray(W_w, dtype=np.float32),
        "w_b": np.ascontiguousarray(W_b, dtype=np.float32).reshape(1, A),
        "conv_w": np.ascontiguousarray(conv_w, dtype=np.float32).reshape(A, A * K),
        "conv_b": np.ascontiguousarray(conv_b, dtype=np.float32).reshape(1, A),
        "x_idx": xi,
        "x_row": xs.astype(np.int32),
    }


def kernel(x, emb_weight, T, W_w, W_b, conv_w, conv_b):
    from concourse.bass_utils import run_bass_kernel_spmd

    nc = _get_nc()
    in_maps = [
        _prep_in_map(x, emb_weight, T, W_w, W_b, conv_w, conv_b, c)
        for c in range(N_CORES)
    ]
    res = run_bass_kernel_spmd(nc, in_maps, core_ids=list(range(N_CORES)))
    outs = res.results
    p_t = np.concatenate([outs[c]["p_t"] for c in range(N_CORES)], axis=0)
    z_s = np.concatenate([outs[c]["z_s"] for c in range(N_CORES)], axis=0)
    r_s = np.concatenate([outs[c]["r_s"] for c in range(N_CORES)], axis=0)
    a_i = np.concatenate([outs[c]["a_i"] for c in range(N_CORES)], axis=0)
    p_t_ = np.concatenate([outs[c]["p_t_"] for c in range(N_CORES)], axis=0)
    return (p_t, z_s, r_s, a_i, p_t_)


# revision 20
# speedup vs baseline: 1.1033x; 1.1033x over previous
"""VQ-codebook model on 8 Trainium2 NeuronCores (Bass/Tile), data-parallel over batch.

Per core (64 batches): embedding gather (indirect DMA) -> cosine-sim vs codebook
(TensorE matmuls, norms folded into operands) -> Conv1d(A->A, k=51) as 51x2
shifted matmul accumulations in PSUM (bf16 weights/acts, fp32 accum) -> ReLU +
channel-max (partition max-tree) -> masked softmax over sequence -> z_s matvecs
-> p_t_ / p_t / r_s small matmuls. No collectives needed (pure data parallel).
"""

import sys
import numpy as np
import ml_dtypes

for _p in ("/opt/trn_rl_repo", "/root/.axon_site/_ro/trn_rl_repo"):
    if _p not in sys.path:
        sys.path.append(_p)

B, S, E, A, V, K = 512, 200, 512, 256, 50000, 51
PAD = 1
NEG_INF = -1e13
N_CORES = 8
B_LOC = B // N_CORES          # 64 batches per core
BG = 4                        # batches per pipelined group
N_GROUPS = B_LOC // BG        # 16
HALF_K = K // 2               # 25
SPAD = S + 2 * HALF_K         # 250 padded sequence length in g buffer

_CACHE = {}


def _build(n_groups=N_GROUPS):
    import concourse.bass as bass
    import concourse.bacc as bacc
    import concourse.mybir as mybir
    import concourse.tile as tile
    from concourse.masks import make_identity
    from concourse import bass_isa

    dt = mybir.dt
    f32 = dt.float32
    f32r = dt.float32r
    bf16 = dt.bfloat16
    i32 = dt.int32
    AX = mybir.AxisListType
    AF = mybir.ActivationFunctionType
    OP = mybir.AluOpType

    nc = bacc.Bacc(None, target_bir_lowering=False)

    emb_d = nc.dram_tensor("emb", [V, E], f32, kind="ExternalInput")
    t_d = nc.dram_tensor("t_mat", [A, E], f32, kind="ExternalInput")
    ww_d = nc.dram_tensor("w_w", [A, E], f32, kind="ExternalInput")
    wb_d = nc.dram_tensor("w_b", [1, A], f32, kind="ExternalInput")
    cwt_d = nc.dram_tensor("conv_wt", [2, 128, K, A], bf16, kind="ExternalInput")
    cb_d = nc.dram_tensor("conv_b", [1, A], f32, kind="ExternalInput")
    # x in two layouts, prepared on host (layout prep only):
    #   xidx[p, h, b] = x[b, h*128 + p]   (gather indices, token-on-partition)
    #   xrow[b, s]    = x[b, s]           (mask rows, batch-on-partition)
    xidx_d = nc.dram_tensor("x_idx", [128, 2, B_LOC], i32, kind="ExternalInput")
    xrow_d = nc.dram_tensor("x_row", [B_LOC, S], i32, kind="ExternalInput")

    pt_d = nc.dram_tensor("p_t", [B_LOC, A], f32, kind="ExternalOutput")
    zs_d = nc.dram_tensor("z_s", [B_LOC, E], f32, kind="ExternalOutput")
    rs_d = nc.dram_tensor("r_s", [B_LOC, E], f32, kind="ExternalOutput")
    ai_d = nc.dram_tensor("a_i", [B_LOC, S], f32, kind="ExternalOutput")
    ptl_d = nc.dram_tensor("p_t_", [B_LOC, A], f32, kind="ExternalOutput")

    with tile.TileContext(nc) as tc:
        with (
            tc.tile_pool(name="pers", bufs=1) as pers,
            tc.tile_pool(name="stage", bufs=1) as stage,
            tc.tile_pool(name="xx", bufs=5) as xxp,
            tc.tile_pool(name="scr", bufs=2) as scr,
            tc.tile_pool(name="xxt", bufs=2) as xxtp,
            tc.tile_pool(name="gbuf", bufs=2) as gbp,
            tc.tile_pool(name="small", bufs=2) as smp,
            tc.tile_pool(name="ppsum", bufs=2, space="PSUM") as ppsum,
            tc.tile_pool(name="mpsum", bufs=2, space="PSUM") as mpsum,
            tc.tile_pool(name="tpsum", bufs=2, space="PSUM") as tpsum,
        ):
            # ---------------- init: constants, weights, transposes -------------
            ident = pers.tile([128, 128], f32)
            make_identity(nc, ident[:])
            identb = pers.tile([128, 128], bf16)
            nc.vector.tensor_copy(identb[:], ident[:])

            ones_row = pers.tile([1, 128], f32)
            nc.vector.memset(ones_row[:], 1.0)
            neg_row = pers.tile([2, S], f32)
            nc.vector.memset(neg_row[:], NEG_INF)
            eps_col = pers.tile([128, 1], f32)
            nc.vector.memset(eps_col[:], 1e-26)

            # x index / row loads
            xidx = pers.tile([128, 2, B_LOC], i32)
            nc.sync.dma_start(out=xidx[:], in_=xidx_d[:])


            # conv bias as per-partition scalars for the two o-chunks
            cb_sb = pers.tile([128, 2], f32)
            nc.sync.dma_start(
                out=cb_sb[:], in_=cb_d[:].rearrange("o (h p) -> (o p) h", h=2, p=128)
            )

            # W_b broadcast to all partitions via rank-1 matmul
            wb_row = pers.tile([1, A], f32)
            nc.sync.dma_start(out=wb_row[:], in_=wb_d[:])
            wb_ps = tpsum.tile([128, A], f32, tag="tp")
            nc.tensor.matmul(wb_ps[:], ones_row[:], wb_row[:], start=True, stop=True)
            wb_bc = pers.tile([128, A], f32)
            nc.vector.tensor_copy(wb_bc[:], wb_ps[:])

            # T rows (unscaled, rhs of r_s) + normalized-transposed T for cosine
            t_rows = pers.tile([128, 2, E], f32)
            nc.sync.dma_start(
                out=t_rows[:], in_=t_d[:].rearrange("(c p) e -> p c e", c=2, p=128)
            )
            t_t = pers.tile([128, 4, A], bf16)  # [e-part, e-chunk, a] scaled by 1/|T_a|
            sq_scr = stage.tile([128, E], f32, tag="sqscr")
            for c in range(2):
                n2 = smp.tile([128, 1], f32, tag="n2")
                nc.scalar.activation(sq_scr[:], t_rows[:, c, :], AF.Square, accum_out=n2[:])
                nrm = smp.tile([128, 1], f32, tag="nrm")
                nc.scalar.activation(nrm[:], n2[:], AF.Sqrt, bias=eps_col[:])
                inv = smp.tile([128, 1], f32, tag="inv")
                nc.vector.reciprocal(inv[:], nrm[:])
                t_nrm = stage.tile([128, E], bf16, tag="tnrm")
                nc.vector.tensor_scalar_mul(t_nrm[:], t_rows[:, c, :], inv[:])
                for ec in range(4):
                    psb = tpsum.tile([128, 128], bf16, tag="tp")
                    nc.tensor.transpose(psb[:], t_nrm[:, ec * 128:(ec + 1) * 128], identb[:])
                    nc.vector.tensor_copy(t_t[:, ec, c * 128:(c + 1) * 128], psb[:])

            # W_w transposed [e, a]
            ww_t = pers.tile([128, 4, A], f32)
            for c in range(2):
                ww_rows = stage.tile([128, E], f32, tag="wwrow")
                nc.sync.dma_start(out=ww_rows[:], in_=ww_d[c * 128:(c + 1) * 128, :])
                for ec in range(4):
                    ps = tpsum.tile([128, 128], f32, tag="tp")
                    nc.tensor.transpose(ps[:], ww_rows[:, ec * 128:(ec + 1) * 128], ident[:])
                    nc.vector.tensor_copy(ww_t[:, ec, c * 128:(c + 1) * 128], ps[:])

            # conv weights arrive pre-transposed [ic, i_local, k, o] in bf16
            cw_t = [pers.tile([128, K, A], bf16, name=f"cw_t{i}", tag=f"cw_t{i}") for i in range(2)]
            for ic in range(2):
                nc.sync.dma_start(out=cw_t[ic][:], in_=cwt_d[ic])

            # persistent accumulator for z (needed on-chip by the tail)
            z_all = pers.tile([B_LOC, E], f32)
            nc.vector.memset(z_all[:], 0.0)

            # ---------------- main loop over groups of BG batches ---------------
            for gi in range(n_groups):
                # group g buffer (conv input), bf16, zero margins
                g_g = [gbp.tile([128, BG, SPAD], bf16, name=f"g_{ic}", tag=f"g{ic}") for ic in range(2)]
                for ic in range(2):
                    nc.vector.memset(g_g[ic][:, :, 0:HALF_K], 0.0)
                    nc.vector.memset(g_g[ic][:, :, S + HALF_K:SPAD], 0.0)

                xxt = xxtp.tile([128, 4, BG * S], bf16, tag="xxt")
                xx_b = []
                for bi in range(BG):
                    b = gi * BG + bi
                    # gather embedding rows: tokens [b*200, b*200+200)
                    xx = xxp.tile([128, 2 * E], f32, tag="xx")
                    nc.gpsimd.indirect_dma_start(
                        out=xx[:, 0:E], out_offset=None, in_=emb_d[:],
                        in_offset=bass.IndirectOffsetOnAxis(ap=xidx[:, 0, b:b + 1], axis=0),
                    )
                    nc.gpsimd.indirect_dma_start(
                        out=xx[0:72, E:2 * E], out_offset=None, in_=emb_d[:],
                        in_offset=bass.IndirectOffsetOnAxis(ap=xidx[0:72, 1, b:b + 1], axis=0),
                    )
                    xxb = xxp.tile([128, 2 * E], bf16, tag="xxb")
                    xx_b.append(xxb)
                    nc.vector.tensor_copy(xxb[:, 0:E], xx[:, 0:E])
                    nc.vector.tensor_copy(xxb[0:72, E:2 * E], xx[0:72, E:2 * E])
                    # token norms -> 1/|xx_s| ; scale rows, then transpose per e-chunk
                    for h, np_ in ((0, 128), (1, 72)):
                        sq = scr.tile([128, E], f32, tag="sq")
                        n2 = smp.tile([128, 1], f32, tag="n2")
                        nc.vector.tensor_tensor(
                            sq[0:np_, :], xx[0:np_, h * E:(h + 1) * E],
                            xx[0:np_, h * E:(h + 1) * E], op=OP.mult,
                        )
                        nc.vector.tensor_reduce(
                            n2[0:np_, :], sq[0:np_, :], axis=AX.X, op=OP.add,
                        )
                        nrm = smp.tile([128, 1], f32, tag="nrm")
                        nc.scalar.activation(nrm[0:np_, :], n2[0:np_, :], AF.Sqrt, bias=eps_col[0:np_, :])
                        inv = smp.tile([128, 1], f32, tag="inv")
                        nc.vector.reciprocal(inv[0:np_, :], nrm[0:np_, :])
                        xs = scr.tile([128, E], bf16, tag="xs")
                        nc.vector.tensor_scalar_mul(
                            xs[0:np_, :], xx[0:np_, h * E:(h + 1) * E], inv[0:np_, :]
                        )
                        for ec in range(4):
                            psb = tpsum.tile([128, 128], bf16, tag="tp")
                            nc.tensor.transpose(
                                psb[0:128, 0:np_], xs[0:np_, ec * 128:(ec + 1) * 128],
                                identb[0:np_, 0:np_],
                            )
                            nc.vector.tensor_copy(
                                xxt[:, ec, bi * S + h * 128: bi * S + h * 128 + np_],
                                psb[:, 0:np_],
                            )

                # cosine similarity -> normalized g, cast to bf16 into g buffer
                for pair in range(2):
                    for ac in range(2):
                        ps = mpsum.tile([128, 2, S], f32, tag="mm")
                        for ec in range(4):
                            nc.tensor.matmul(
                                ps[:],
                                t_t[:, ec, ac * 128:(ac + 1) * 128],
                                xxt[:, ec, pair * 2 * S:(pair + 1) * 2 * S].rearrange(
                                    "p (b s) -> p b s", b=2, s=S
                                ),
                                start=(ec == 0), stop=(ec == 3),
                            )
                        nc.vector.tensor_copy(
                            g_g[ac][:, pair * 2:(pair + 1) * 2, HALF_K:HALF_K + S], ps[:]
                        )

                # conv + post-processing, pair-sequential so psum slots
                # double-buffer across pairs/groups (keeps PE stream dense)
                for pair in range(2):
                    cpsp = [ppsum.tile([128, 2, S], f32, name=f"cps{oc}", tag=f"cv{oc}")
                            for oc in range(2)]
                    for oc in range(2):
                        for k in range(K):
                            for ic in range(2):
                                nc.tensor.matmul(
                                    cpsp[oc][:],
                                    cw_t[ic][:, k, oc * 128:(oc + 1) * 128],
                                    g_g[ic][:, pair * 2:(pair + 1) * 2, k:k + S],
                                    start=(k == 0 and ic == 0),
                                    stop=(k == K - 1 and ic == 1),
                                )
                    u0 = smp.tile([128, 2, S], f32, tag="u0")
                    u1 = smp.tile([128, 2, S], f32, tag="u1")
                    nc.scalar.activation(u0[:], cpsp[0][:], AF.Relu, bias=cb_sb[:, 0:1])
                    nc.scalar.activation(u1[:], cpsp[1][:], AF.Relu, bias=cb_sb[:, 1:2])
                    # channel max: combine the two o-chunks elementwise (same
                    # base partition), then transpose to [s, o] and reduce free
                    nc.vector.tensor_max(u0[:], u0[:], u1[:])
                    ut = smp.tile([128, 2, 2, 128], f32, tag="ut")
                    for j in range(2):
                        for h, sb in ((0, 128), (1, 72)):
                            ps = tpsum.tile([128, 128], f32, tag="tp")
                            nc.tensor.transpose(
                                ps[0:sb, 0:128], u0[:, j, h * 128:h * 128 + sb],
                                ident[:],
                            )
                            nc.vector.tensor_copy(ut[0:sb, h, j, :], ps[0:sb, :])
                    mcols = smp.tile([128, 2, 2], f32, tag="mcols")
                    for j in range(2):
                        for h, sb in ((0, 128), (1, 72)):
                            nc.vector.tensor_reduce(
                                mcols[0:sb, h, j:j + 1], ut[0:sb, h, j, :],
                                axis=AX.X, op=OP.max,
                            )
                    mg = smp.tile([2, S], f32, tag="mg")
                    psm0 = tpsum.tile([128, 128], f32, tag="tp")
                    nc.tensor.transpose(psm0[0:2, 0:128], mcols[:, 0, :], ident[:])
                    nc.vector.tensor_copy(mg[:, 0:128], psm0[0:2, 0:128])
                    psm1 = tpsum.tile([128, 128], f32, tag="tp")
                    nc.tensor.transpose(psm1[0:2, 0:72], mcols[0:72, 1, :], ident[0:72, 0:72])
                    nc.vector.tensor_copy(mg[:, 128:S], psm1[0:2, 0:72])

                    # masked softmax over s for the two batches (rows 0-1)
                    b0 = gi * BG + pair * 2
                    xfi = smp.tile([2, S], i32, tag="xfi")
                    nc.sync.dma_start(out=xfi[:], in_=xrow_d[b0:b0 + 2, :])
                    msk = smp.tile([2, S], dt.uint8, tag="msk")
                    nc.vector.tensor_scalar(
                        msk[:], xfi[:], PAD, None, op0=OP.is_equal
                    )
                    nc.vector.copy_predicated(mg[:], msk[:], neg_row[0:2, :])
                    rmx = smp.tile([2, 1], f32, tag="rmx")
                    nc.vector.tensor_reduce(
                        rmx[:], mg[:], axis=AX.X, op=OP.max, negate=True
                    )
                    em = smp.tile([2, S], f32, tag="em")
                    nc.scalar.activation(em[:], mg[:], AF.Exp, bias=rmx[:])
                    rsm = smp.tile([2, 1], f32, tag="rsm")
                    nc.vector.tensor_reduce(rsm[:], em[:], axis=AX.X, op=OP.add)
                    rin = smp.tile([2, 1], f32, tag="rin")
                    nc.vector.reciprocal(rin[:], rsm[:])
                    ag = smp.tile([2, S], f32, tag="ag")
                    nc.vector.tensor_scalar_mul(ag[:], em[:], rin[:])
                    nc.sync.dma_start(out=ai_d[b0:b0 + 2, :], in_=ag[:])

                    # a_i columns for z_s matvecs
                    at = smp.tile([128, 2, 2], bf16, tag="at")
                    psa0 = tpsum.tile([128, 128], f32, tag="tp")
                    nc.tensor.transpose(psa0[0:128, 0:2], ag[:, 0:128], ident[0:2, 0:2])
                    nc.vector.tensor_copy(at[:, 0, :], psa0[:, 0:2])
                    psa1 = tpsum.tile([128, 128], f32, tag="tp")
                    nc.tensor.transpose(psa1[0:72, 0:2], ag[:, 128:S], ident[0:2, 0:2])
                    nc.vector.tensor_copy(at[0:72, 1, :], psa1[0:72, 0:2])

                    for j in range(2):
                        bi = pair * 2 + j
                        b = gi * BG + bi
                        zp = mpsum.tile([1, E], f32, tag="mm")
                        nc.tensor.matmul(
                            zp[:], at[:, 0, j:j + 1], xx_b[bi][:, 0:E],
                            start=True, stop=False,
                        )
                        nc.tensor.matmul(
                            zp[:], at[0:72, 1, j:j + 1], xx_b[bi][0:72, E:2 * E],
                            start=False, stop=True,
                        )
                        zrow = smp.tile([1, E], f32, tag="zrow")
                        nc.vector.tensor_copy(zrow[:], zp[:])
                        nc.sync.dma_start(out=z_all[b:b + 1, :], in_=zrow[:])

            # ---------------- tail: p_t_, p_t, r_s -----------------------------
            z_t = pers.tile([128, 4, B_LOC], f32)
            for ec in range(4):
                ps = tpsum.tile([128, 128], f32, tag="tp")
                nc.tensor.transpose(
                    ps[0:128, 0:B_LOC], z_all[:, ec * 128:(ec + 1) * 128],
                    ident[0:B_LOC, 0:B_LOC],
                )
                nc.vector.tensor_copy(z_t[:, ec, :], ps[:, 0:B_LOC])
            pp = mpsum.tile([B_LOC, A], f32, tag="mm")
            for ec in range(4):
                nc.tensor.matmul(
                    pp[:], z_t[:, ec, :], ww_t[:, ec, :],
                    start=(ec == 0), stop=(ec == 3),
                )
            ptl = pers.tile([B_LOC, A], f32)
            nc.vector.tensor_add(ptl[:], pp[:], wb_bc[0:B_LOC, :])
            rmx = pers.tile([B_LOC, 1], f32)
            nc.vector.tensor_reduce(rmx[:], ptl[:], axis=AX.X, op=OP.max, negate=True)
            em = pers.tile([B_LOC, A], f32)
            nc.scalar.activation(em[:], ptl[:], AF.Exp, bias=rmx[:])
            rsm = pers.tile([B_LOC, 1], f32)
            nc.vector.tensor_reduce(rsm[:], em[:], axis=AX.X, op=OP.add)
            rin = pers.tile([B_LOC, 1], f32)
            nc.vector.reciprocal(rin[:], rsm[:])
            ptp = pers.tile([B_LOC, A], f32)
            nc.vector.tensor_scalar_mul(ptp[:], em[:], rin[:])

            pt_t = pers.tile([128, 2, B_LOC], f32)
            for ac in range(2):
                ps = tpsum.tile([128, 128], f32, tag="tp")
                nc.tensor.transpose(
                    ps[0:128, 0:B_LOC], ptp[:, ac * 128:(ac + 1) * 128],
                    ident[0:B_LOC, 0:B_LOC],
                )
                nc.vector.tensor_copy(pt_t[:, ac, :], ps[:, 0:B_LOC])
            rp = mpsum.tile([B_LOC, E], f32, tag="mm")
            for ac in range(2):
                nc.tensor.matmul(
                    rp[:], pt_t[:, ac, :], t_rows[:, ac, :],
                    start=(ac == 0), stop=(ac == 1),
                )
            rsb = pers.tile([B_LOC, E], f32)
            nc.vector.tensor_copy(rsb[:], rp[:])

            nc.sync.dma_start(out=pt_d[:], in_=ptp[:])
            nc.sync.dma_start(out=zs_d[:], in_=z_all[:])
            nc.sync.dma_start(out=rs_d[:], in_=rsb[:])
            nc.sync.dma_start(out=ptl_d[:], in_=ptl[:])

    nc.compile()
    return nc


def _get_nc():
    if "nc" not in _CACHE:
        _CACHE["nc"] = _build()
    return _CACHE["nc"]


def _prep_in_map(x, emb_weight, T, W_w, W_b, conv_w, conv_b, core):
    xs = np.asarray(x[core * B_LOC:(core + 1) * B_LOC], dtype=np.int64)
    xi = np.zeros((128, 2, B_LOC), dtype=np.int32)
    xi[:, 0, :] = xs[:, 0:128].T.astype(np.int32)
    xi[0:72, 1, :] = xs[:, 128:200].T.astype(np.int32)
    return {
        "emb": np.ascontiguousarray(emb_weight, dtype=np.float32),
        "t_mat": np.ascontiguousarray(T, dtype=np.float32),
        "w_w": np.ascontiguousarray(W_w, dtype=np.float32),
        "w_b": np.ascontiguousarray(W_b, dtype=np.float32).reshape(1, A),
        "conv_wt": np.ascontiguousarray(
            np.asarray(conv_w, dtype=np.float32).reshape(A, A, K)
            .transpose(1, 2, 0).reshape(2, 128, K, A)
            .astype(ml_dtypes.bfloat16)),
        "conv_b": np.ascontiguousarray(conv_b, dtype=np.float32).reshape(1, A),
        "x_idx": xi,
        "x_row": xs.astype(np.int32),
    }


def kernel(x, emb_weight, T, W_w, W_b, conv_w, conv_b):
    from concourse.bass_utils import run_bass_kernel_spmd

    nc = _get_nc()
    in_maps = [
        _prep_in_map(x, emb_weight, T, W_w, W_b, conv_w, conv_b, c)
        for c in range(N_CORES)
    ]
    res = run_bass_kernel_spmd(nc, in_maps, core_ids=list(range(N_CORES)))
    outs = res.results
    p_t = np.concatenate([outs[c]["p_t"] for c in range(N_CORES)], axis=0)
    z_s = np.concatenate([outs[c]["z_s"] for c in range(N_CORES)], axis=0)
    r_s = np.concatenate([outs[c]["r_s"] for c in range(N_CORES)], axis=0)
    a_i = np.concatenate([outs[c]["a_i"] for c in range(N_CORES)], axis=0)
    p_t_ = np.concatenate([outs[c]["p_t_"] for c in range(N_CORES)], axis=0)
    return (p_t, z_s, r_s, a_i, p_t_)


# revision 21
# speedup vs baseline: 1.3228x; 1.1990x over previous
"""VQ-codebook model on 8 Trainium2 NeuronCores (Bass/Tile), data-parallel over batch.

Per core (64 batches): embedding gather (indirect DMA) -> cosine-sim vs codebook
(TensorE matmuls, norms folded into operands) -> Conv1d(A->A, k=51) as 51x2
shifted matmul accumulations in PSUM (bf16 weights/acts, fp32 accum) -> ReLU +
channel-max (partition max-tree) -> masked softmax over sequence -> z_s matvecs
-> p_t_ / p_t / r_s small matmuls. No collectives needed (pure data parallel).
"""

import sys
import numpy as np
import ml_dtypes

for _p in ("/opt/trn_rl_repo", "/root/.axon_site/_ro/trn_rl_repo"):
    if _p not in sys.path:
        sys.path.append(_p)

B, S, E, A, V, K = 512, 200, 512, 256, 50000, 51
PAD = 1
NEG_INF = -1e13
N_CORES = 8
B_LOC = B // N_CORES          # 64 batches per core
BG = 4                        # batches per pipelined group
N_GROUPS = B_LOC // BG        # 16
HALF_K = K // 2               # 25
SPAD = S + 2 * HALF_K         # 250 padded sequence length in g buffer

_CACHE = {}


def _build(n_groups=N_GROUPS):
    import concourse.bass as bass
    import concourse.bacc as bacc
    import concourse.mybir as mybir
    import concourse.tile as tile
    from concourse.masks import make_identity
    from concourse import bass_isa

    dt = mybir.dt
    f32 = dt.float32
    f32r = dt.float32r
    bf16 = dt.bfloat16
    i32 = dt.int32
    AX = mybir.AxisListType
    AF = mybir.ActivationFunctionType
    OP = mybir.AluOpType

    nc = bacc.Bacc(None, target_bir_lowering=False)

    emb_d = nc.dram_tensor("emb", [V, E], f32, kind="ExternalInput")
    t_d = nc.dram_tensor("t_mat", [A, E], f32, kind="ExternalInput")
    ww_d = nc.dram_tensor("w_w", [A, E], f32, kind="ExternalInput")
    wb_d = nc.dram_tensor("w_b", [1, A], f32, kind="ExternalInput")
    cwt_d = nc.dram_tensor("conv_wt", [2, 128, K, A], bf16, kind="ExternalInput")
    cb_d = nc.dram_tensor("conv_b", [1, A], f32, kind="ExternalInput")
    # x in two layouts, prepared on host (layout prep only):
    #   xidx[p, h, b] = x[b, h*128 + p]   (gather indices, token-on-partition)
    #   xrow[b, s]    = x[b, s]           (mask rows, batch-on-partition)
    xidx_d = nc.dram_tensor("x_idx", [128, 2, B_LOC], i32, kind="ExternalInput")
    xrow_d = nc.dram_tensor("x_row", [B_LOC, S], i32, kind="ExternalInput")

    pt_d = nc.dram_tensor("p_t", [B_LOC, A], f32, kind="ExternalOutput")
    zs_d = nc.dram_tensor("z_s", [B_LOC, E], f32, kind="ExternalOutput")
    rs_d = nc.dram_tensor("r_s", [B_LOC, E], f32, kind="ExternalOutput")
    ai_d = nc.dram_tensor("a_i", [B_LOC, S], f32, kind="ExternalOutput")
    ptl_d = nc.dram_tensor("p_t_", [B_LOC, A], f32, kind="ExternalOutput")

    with tile.TileContext(nc) as tc:
        with (
            tc.tile_pool(name="pers", bufs=1) as pers,
            tc.tile_pool(name="stage", bufs=1) as stage,
            tc.tile_pool(name="xx", bufs=5) as xxp,
            tc.tile_pool(name="scr", bufs=2) as scr,
            tc.tile_pool(name="xxt", bufs=2) as xxtp,
            tc.tile_pool(name="gbuf", bufs=2) as gbp,
            tc.tile_pool(name="small", bufs=2) as smp,
            tc.tile_pool(name="ppsum", bufs=2, space="PSUM") as ppsum,
            tc.tile_pool(name="mpsum", bufs=2, space="PSUM") as mpsum,
            tc.tile_pool(name="tpsum", bufs=2, space="PSUM") as tpsum,
        ):
            # ---------------- init: constants, weights, transposes -------------
            ident = pers.tile([128, 128], f32)
            make_identity(nc, ident[:])
            identb = pers.tile([128, 128], bf16)
            nc.vector.tensor_copy(identb[:], ident[:])

            ones_row = pers.tile([1, 128], f32)
            nc.vector.memset(ones_row[:], 1.0)
            neg_row = pers.tile([2, S], f32)
            nc.vector.memset(neg_row[:], NEG_INF)
            eps_col = pers.tile([128, 1], f32)
            nc.vector.memset(eps_col[:], 1e-26)

            # x index / row loads
            xidx = pers.tile([128, 2, B_LOC], i32)
            nc.sync.dma_start(out=xidx[:], in_=xidx_d[:])


            # conv bias as per-partition scalars for the two o-chunks
            cb_sb = pers.tile([128, 2], f32)
            nc.sync.dma_start(
                out=cb_sb[:], in_=cb_d[:].rearrange("o (h p) -> (o p) h", h=2, p=128)
            )

            # W_b broadcast to all partitions via rank-1 matmul
            wb_row = pers.tile([1, A], f32)
            nc.sync.dma_start(out=wb_row[:], in_=wb_d[:])
            wb_ps = tpsum.tile([128, A], f32, tag="tp")
            nc.tensor.matmul(wb_ps[:], ones_row[:], wb_row[:], start=True, stop=True)
            wb_bc = pers.tile([128, A], f32)
            nc.vector.tensor_copy(wb_bc[:], wb_ps[:])

            # T rows (unscaled, rhs of r_s) + normalized-transposed T for cosine
            t_rows = pers.tile([128, 2, E], f32)
            nc.sync.dma_start(
                out=t_rows[:], in_=t_d[:].rearrange("(c p) e -> p c e", c=2, p=128)
            )
            t_t = pers.tile([128, 4, A], bf16)  # [e-part, e-chunk, a] scaled by 1/|T_a|
            sq_scr = stage.tile([128, E], f32, tag="sqscr")
            for c in range(2):
                n2 = smp.tile([128, 1], f32, tag="n2")
                nc.scalar.activation(sq_scr[:], t_rows[:, c, :], AF.Square, accum_out=n2[:])
                nrm = smp.tile([128, 1], f32, tag="nrm")
                nc.scalar.activation(nrm[:], n2[:], AF.Sqrt, bias=eps_col[:])
                inv = smp.tile([128, 1], f32, tag="inv")
                nc.vector.reciprocal(inv[:], nrm[:])
                t_nrm = stage.tile([128, E], bf16, tag="tnrm")
                nc.vector.tensor_scalar_mul(t_nrm[:], t_rows[:, c, :], inv[:])
                for ec in range(4):
                    psb = tpsum.tile([128, 128], bf16, tag="tp")
                    nc.tensor.transpose(psb[:], t_nrm[:, ec * 128:(ec + 1) * 128], identb[:])
                    nc.vector.tensor_copy(t_t[:, ec, c * 128:(c + 1) * 128], psb[:])

            # W_w transposed [e, a]
            ww_t = pers.tile([128, 4, A], f32)
            for c in range(2):
                ww_rows = stage.tile([128, E], f32, tag="wwrow")
                nc.sync.dma_start(out=ww_rows[:], in_=ww_d[c * 128:(c + 1) * 128, :])
                for ec in range(4):
                    ps = tpsum.tile([128, 128], f32, tag="tp")
                    nc.tensor.transpose(ps[:], ww_rows[:, ec * 128:(ec + 1) * 128], ident[:])
                    nc.vector.tensor_copy(ww_t[:, ec, c * 128:(c + 1) * 128], ps[:])

            # conv weights arrive pre-transposed [ic, i_local, k, o] in bf16
            cw_t = [pers.tile([128, K, A], bf16, name=f"cw_t{i}", tag=f"cw_t{i}") for i in range(2)]
            for ic in range(2):
                for kc in range(4):
                    k0, k1 = kc * 13, min(K, (kc + 1) * 13)
                    nc.sync.dma_start(
                        out=cw_t[ic][:, k0:k1, :], in_=cwt_d[ic, :, k0:k1, :]
                    )

            # persistent accumulator for z (needed on-chip by the tail)
            z_all = pers.tile([B_LOC, E], f32)
            nc.vector.memset(z_all[:], 0.0)

            # ---------------- main loop over groups of BG batches ---------------
            for gi in range(n_groups):
                # group g buffer (conv input), bf16, zero margins
                g_g = [gbp.tile([128, BG, SPAD], bf16, name=f"g_{ic}", tag=f"g{ic}") for ic in range(2)]
                for ic in range(2):
                    nc.vector.memset(g_g[ic][:, :, 0:HALF_K], 0.0)
                    nc.vector.memset(g_g[ic][:, :, S + HALF_K:SPAD], 0.0)

                xxt = xxtp.tile([128, 4, BG * S], bf16, tag="xxt")
                xx_b = []
                for bi in range(BG):
                    b = gi * BG + bi
                    # gather embedding rows: tokens [b*200, b*200+200)
                    xx = xxp.tile([128, 2 * E], f32, tag="xx")
                    nc.gpsimd.indirect_dma_start(
                        out=xx[:, 0:E], out_offset=None, in_=emb_d[:],
                        in_offset=bass.IndirectOffsetOnAxis(ap=xidx[:, 0, b:b + 1], axis=0),
                    )
                    nc.gpsimd.indirect_dma_start(
                        out=xx[0:72, E:2 * E], out_offset=None, in_=emb_d[:],
                        in_offset=bass.IndirectOffsetOnAxis(ap=xidx[0:72, 1, b:b + 1], axis=0),
                    )
                    xxb = xxp.tile([128, 2 * E], bf16, tag="xxb")
                    xx_b.append(xxb)
                    nc.vector.tensor_copy(xxb[:, 0:E], xx[:, 0:E])
                    nc.vector.tensor_copy(xxb[0:72, E:2 * E], xx[0:72, E:2 * E])
                    # token norms -> 1/|xx_s| ; scale rows, then transpose per e-chunk
                    for h, np_ in ((0, 128), (1, 72)):
                        sq = scr.tile([128, E], f32, tag="sq")
                        n2 = smp.tile([128, 1], f32, tag="n2")
                        nc.vector.tensor_tensor(
                            sq[0:np_, :], xx[0:np_, h * E:(h + 1) * E],
                            xx[0:np_, h * E:(h + 1) * E], op=OP.mult,
                        )
                        nc.vector.tensor_reduce(
                            n2[0:np_, :], sq[0:np_, :], axis=AX.X, op=OP.add,
                        )
                        nrm = smp.tile([128, 1], f32, tag="nrm")
                        nc.scalar.activation(nrm[0:np_, :], n2[0:np_, :], AF.Sqrt, bias=eps_col[0:np_, :])
                        inv = smp.tile([128, 1], f32, tag="inv")
                        nc.vector.reciprocal(inv[0:np_, :], nrm[0:np_, :])
                        xs = scr.tile([128, E], bf16, tag="xs")
                        nc.vector.tensor_scalar_mul(
                            xs[0:np_, :], xx[0:np_, h * E:(h + 1) * E], inv[0:np_, :]
                        )
                        for ec in range(4):
                            psb = tpsum.tile([128, 128], bf16, tag="tp")
                            nc.tensor.transpose(
                                psb[0:128, 0:np_], xs[0:np_, ec * 128:(ec + 1) * 128],
                                identb[0:np_, 0:np_],
                            )
                            nc.vector.tensor_copy(
                                xxt[:, ec, bi * S + h * 128: bi * S + h * 128 + np_],
                                psb[:, 0:np_],
                            )

                # cosine similarity -> normalized g, cast to bf16 into g buffer
                for pair in range(2):
                    for ac in range(2):
                        ps = mpsum.tile([128, 2, S], f32, tag="mm")
                        for ec in range(4):
                            nc.tensor.matmul(
                                ps[:],
                                t_t[:, ec, ac * 128:(ac + 1) * 128],
                                xxt[:, ec, pair * 2 * S:(pair + 1) * 2 * S].rearrange(
                                    "p (b s) -> p b s", b=2, s=S
                                ),
                                start=(ec == 0), stop=(ec == 3),
                            )
                        nc.vector.tensor_copy(
                            g_g[ac][:, pair * 2:(pair + 1) * 2, HALF_K:HALF_K + S], ps[:]
                        )

                # conv + post-processing, pair-sequential so psum slots
                # double-buffer across pairs/groups (keeps PE stream dense)
                for pair in range(2):
                    cpsp = [ppsum.tile([128, 2, S], f32, name=f"cps{oc}", tag=f"cv{oc}")
                            for oc in range(2)]
                    for oc in range(2):
                        for k in range(K):
                            for ic in range(2):
                                nc.tensor.matmul(
                                    cpsp[oc][:],
                                    cw_t[ic][:, k, oc * 128:(oc + 1) * 128],
                                    g_g[ic][:, pair * 2:(pair + 1) * 2, k:k + S],
                                    start=(k == 0 and ic == 0),
                                    stop=(k == K - 1 and ic == 1),
                                )
                    u0 = smp.tile([128, 2, S], f32, tag="u0")
                    u1 = smp.tile([128, 2, S], f32, tag="u1")
                    nc.scalar.activation(u0[:], cpsp[0][:], AF.Relu, bias=cb_sb[:, 0:1])
                    nc.scalar.activation(u1[:], cpsp[1][:], AF.Relu, bias=cb_sb[:, 1:2])
                    # channel max: combine the two o-chunks elementwise (same
                    # base partition), then transpose to [s, o] and reduce free
                    nc.vector.tensor_max(u0[:], u0[:], u1[:])
                    ut = smp.tile([128, 2, 2, 128], f32, tag="ut")
                    for j in range(2):
                        for h, sb in ((0, 128), (1, 72)):
                            ps = tpsum.tile([128, 128], f32, tag="tp")
                            nc.tensor.transpose(
                                ps[0:sb, 0:128], u0[:, j, h * 128:h * 128 + sb],
                                ident[:],
                            )
                            nc.vector.tensor_copy(ut[0:sb, h, j, :], ps[0:sb, :])
                    mcols = smp.tile([128, 2, 2], f32, tag="mcols")
                    for j in range(2):
                        for h, sb in ((0, 128), (1, 72)):
                            nc.vector.tensor_reduce(
                                mcols[0:sb, h, j:j + 1], ut[0:sb, h, j, :],
                                axis=AX.X, op=OP.max,
                            )
                    mg = smp.tile([2, S], f32, tag="mg")
                    psm0 = tpsum.tile([128, 128], f32, tag="tp")
                    nc.tensor.transpose(psm0[0:2, 0:128], mcols[:, 0, :], ident[:])
                    nc.vector.tensor_copy(mg[:, 0:128], psm0[0:2, 0:128])
                    psm1 = tpsum.tile([128, 128], f32, tag="tp")
                    nc.tensor.transpose(psm1[0:2, 0:72], mcols[0:72, 1, :], ident[0:72, 0:72])
                    nc.vector.tensor_copy(mg[:, 128:S], psm1[0:2, 0:72])

                    # masked softmax over s for the two batches (rows 0-1)
                    b0 = gi * BG + pair * 2
                    xfi = smp.tile([2, S], i32, tag="xfi")
                    nc.sync.dma_start(out=xfi[:], in_=xrow_d[b0:b0 + 2, :])
                    msk = smp.tile([2, S], dt.uint8, tag="msk")
                    nc.vector.tensor_scalar(
                        msk[:], xfi[:], PAD, None, op0=OP.is_equal
                    )
                    nc.vector.copy_predicated(mg[:], msk[:], neg_row[0:2, :])
                    rmx = smp.tile([2, 1], f32, tag="rmx")
                    nc.vector.tensor_reduce(
                        rmx[:], mg[:], axis=AX.X, op=OP.max, negate=True
                    )
                    em = smp.tile([2, S], f32, tag="em")
                    nc.scalar.activation(em[:], mg[:], AF.Exp, bias=rmx[:])
                    rsm = smp.tile([2, 1], f32, tag="rsm")
                    nc.vector.tensor_reduce(rsm[:], em[:], axis=AX.X, op=OP.add)
                    rin = smp.tile([2, 1], f32, tag="rin")
                    nc.vector.reciprocal(rin[:], rsm[:])
                    ag = smp.tile([2, S], f32, tag="ag")
                    nc.vector.tensor_scalar_mul(ag[:], em[:], rin[:])
                    nc.sync.dma_start(out=ai_d[b0:b0 + 2, :], in_=ag[:])

                    # a_i columns for z_s matvecs
                    at = smp.tile([128, 2, 2], bf16, tag="at")
                    psa0 = tpsum.tile([128, 128], f32, tag="tp")
                    nc.tensor.transpose(psa0[0:128, 0:2], ag[:, 0:128], ident[0:2, 0:2])
                    nc.vector.tensor_copy(at[:, 0, :], psa0[:, 0:2])
                    psa1 = tpsum.tile([128, 128], f32, tag="tp")
                    nc.tensor.transpose(psa1[0:72, 0:2], ag[:, 128:S], ident[0:2, 0:2])
                    nc.vector.tensor_copy(at[0:72, 1, :], psa1[0:72, 0:2])

                    for j in range(2):
                        bi = pair * 2 + j
                        b = gi * BG + bi
                        zp = mpsum.tile([1, E], f32, tag="mm")
                        nc.tensor.matmul(
                            zp[:], at[:, 0, j:j + 1], xx_b[bi][:, 0:E],
                            start=True, stop=False,
                        )
                        nc.tensor.matmul(
                            zp[:], at[0:72, 1, j:j + 1], xx_b[bi][0:72, E:2 * E],
                            start=False, stop=True,
                        )
                        zrow = smp.tile([1, E], f32, tag="zrow")
                        nc.vector.tensor_copy(zrow[:], zp[:])
                        nc.sync.dma_start(out=z_all[b:b + 1, :], in_=zrow[:])

            # ---------------- tail: p_t_, p_t, r_s -----------------------------
            z_t = pers.tile([128, 4, B_LOC], f32)
            for ec in range(4):
                ps = tpsum.tile([128, 128], f32, tag="tp")
                nc.tensor.transpose(
                    ps[0:128, 0:B_LOC], z_all[:, ec * 128:(ec + 1) * 128],
                    ident[0:B_LOC, 0:B_LOC],
                )
                nc.vector.tensor_copy(z_t[:, ec, :], ps[:, 0:B_LOC])
            pp = mpsum.tile([B_LOC, A], f32, tag="mm")
            for ec in range(4):
                nc.tensor.matmul(
                    pp[:], z_t[:, ec, :], ww_t[:, ec, :],
                    start=(ec == 0), stop=(ec == 3),
                )
            ptl = pers.tile([B_LOC, A], f32)
            nc.vector.tensor_add(ptl[:], pp[:], wb_bc[0:B_LOC, :])
            rmx = pers.tile([B_LOC, 1], f32)
            nc.vector.tensor_reduce(rmx[:], ptl[:], axis=AX.X, op=OP.max, negate=True)
            em = pers.tile([B_LOC, A], f32)
            nc.scalar.activation(em[:], ptl[:], AF.Exp, bias=rmx[:])
            rsm = pers.tile([B_LOC, 1], f32)
            nc.vector.tensor_reduce(rsm[:], em[:], axis=AX.X, op=OP.add)
            rin = pers.tile([B_LOC, 1], f32)
            nc.vector.reciprocal(rin[:], rsm[:])
            ptp = pers.tile([B_LOC, A], f32)
            nc.vector.tensor_scalar_mul(ptp[:], em[:], rin[:])

            pt_t = pers.tile([128, 2, B_LOC], f32)
            for ac in range(2):
                ps = tpsum.tile([128, 128], f32, tag="tp")
                nc.tensor.transpose(
                    ps[0:128, 0:B_LOC], ptp[:, ac * 128:(ac + 1) * 128],
                    ident[0:B_LOC, 0:B_LOC],
                )
                nc.vector.tensor_copy(pt_t[:, ac, :], ps[:, 0:B_LOC])
            rp = mpsum.tile([B_LOC, E], f32, tag="mm")
            for ac in range(2):
                nc.tensor.matmul(
                    rp[:], pt_t[:, ac, :], t_rows[:, ac, :],
                    start=(ac == 0), stop=(ac == 1),
                )
            rsb = pers.tile([B_LOC, E], f32)
            nc.vector.tensor_copy(rsb[:], rp[:])

            nc.sync.dma_start(out=pt_d[:], in_=ptp[:])
            nc.sync.dma_start(out=zs_d[:], in_=z_all[:])
            nc.sync.dma_start(out=rs_d[:], in_=rsb[:])
            nc.sync.dma_start(out=ptl_d[:], in_=ptl[:])

    nc.compile()
    return nc


def _get_nc():
    if "nc" not in _CACHE:
        _CACHE["nc"] = _build()
    return _CACHE["nc"]


def _prep_in_map(x, emb_weight, T, W_w, W_b, conv_w, conv_b, core):
    xs = np.asarray(x[core * B_LOC:(core + 1) * B_LOC], dtype=np.int64)
    xi = np.zeros((128, 2, B_LOC), dtype=np.int32)
    xi[:, 0, :] = xs[:, 0:128].T.astype(np.int32)
    xi[0:72, 1, :] = xs[:, 128:200].T.astype(np.int32)
    return {
        "emb": np.ascontiguousarray(emb_weight, dtype=np.float32),
        "t_mat": np.ascontiguousarray(T, dtype=np.float32),
        "w_w": np.ascontiguousarray(W_w, dtype=np.float32),
        "w_b": np.ascontiguousarray(W_b, dtype=np.float32).reshape(1, A),
        "conv_wt": np.ascontiguousarray(
            np.asarray(conv_w, dtype=np.float32).reshape(A, A, K)
            .transpose(1, 2, 0).reshape(2, 128, K, A)
            .astype(ml_dtypes.bfloat16)),
        "conv_b": np.ascontiguousarray(conv_b, dtype=np.float32).reshape(1, A),
        "x_idx": xi,
        "x_row": xs.astype(np.int32),
    }


def kernel(x, emb_weight, T, W_w, W_b, conv_w, conv_b):
    from concourse.bass_utils import run_bass_kernel_spmd

    nc = _get_nc()
    in_maps = [
        _prep_in_map(x, emb_weight, T, W_w, W_b, conv_w, conv_b, c)
        for c in range(N_CORES)
    ]
    res = run_bass_kernel_spmd(nc, in_maps, core_ids=list(range(N_CORES)))
    outs = res.results
    p_t = np.concatenate([outs[c]["p_t"] for c in range(N_CORES)], axis=0)
    z_s = np.concatenate([outs[c]["z_s"] for c in range(N_CORES)], axis=0)
    r_s = np.concatenate([outs[c]["r_s"] for c in range(N_CORES)], axis=0)
    a_i = np.concatenate([outs[c]["a_i"] for c in range(N_CORES)], axis=0)
    p_t_ = np.concatenate([outs[c]["p_t_"] for c in range(N_CORES)], axis=0)
    return (p_t, z_s, r_s, a_i, p_t_)
